# revision 8
# baseline (speedup 1.0000x reference)
"""Multi-head causal attention (B=4, S=2048, D=1024, H=16) on 8 TRN2 NeuronCores.

Sharding: core c -> (batch b = c//2, head-group g = c%2). Each core computes
8 heads for one batch: QKV projection (tensor-parallel column slice), causal
softmax attention, and a row-parallel slice of the output projection. The two
cores of a batch produce partial outputs that the host sums; biases that
commute with the attention (v bias, out bias) are folded into a single
host-side vector add.

Single fused pipeline per 512-token s-block: QKV(s) -> attention(i-window s)
-> output projection(s). This overlaps the Activation-engine exp work of
window s with the QKV matmuls of window s+1 (the separated-phase version left
ACT idle for the whole QKV phase).

All matmuls run in float32r (11-bit-mantissa fp32, full PE rate for moving
free size >= 256 -- diagonal tiles are widened to >=256 to stay off the
quarter-rate path). Device layout notes:
 - q/k are produced transposed: kT[p] = [128 partitions (2 heads x 64 hd), S];
   q lives in a rotating per-window buffer qw[p] = [128, 512]. scoresT[j, i]
   = kT.T @ qw has keys j on partitions; the two heads of a pair run as
   concurrent row-tiled matmuls (tile_position (0,0)/(64,0) auto-derived from
   base partitions).
 - v is produced in [s, dv] layout with an interleaved ones column per head
   ([v_h | 1], width 65) so attn@v also yields the softmax denominator row.
 - causal handling: off-diagonal j-tiles are full 512-wide matmuls; diagonal
   j-tiles compute width max(256, 512-128r), mask the one triangular 128x128
   boundary block additively, and zero-fill the invalid strip of the exp tile
   on GpSimd (only needed for r=3). The key (padding) mask enters as the
   per-partition bias of the exp activation (0 or -1e30 per key).
 - softmax denominators: reciprocal on DVE, broadcast across partitions on
   GpSimd, one multiply into the normalized attention window buffer.
"""

import numpy as np
from contextlib import ExitStack

B, S, D, H = 4, 2048, 1024, 16
HD = D // H          # 64
HPC = H // 2         # 8 heads per core
DV = HPC * HD        # 512 v-dims per core
N_CORES = 8
SB = 512             # i-tile width (matmul N)
NSB = S // SB        # 4
NJT = S // 128       # 16 j-tiles

_CACHE = {}


def _build_module():
    import os
    KREP = int(os.environ.get("KREP", "1"))
    PSB = int(os.environ.get("PSB", "3"))
    PJB = int(os.environ.get("PJB", "1"))
    PQB = int(os.environ.get("PQB", "2"))
    APSB = int(os.environ.get("APSB", "2"))
    EPB = int(os.environ.get("EPB", "5"))
    XPB = int(os.environ.get("XPB", "8"))
    NPB = int(os.environ.get("NPB", "2"))
    YPB = int(os.environ.get("YPB", "3"))
    QWB = int(os.environ.get("QWB", "2"))
    AWB = int(os.environ.get("AWB", "2"))
    import concourse.bacc as bacc
    import concourse.mybir as mybir
    import concourse.tile as tile
    from concourse._compat import get_trn_type

    F32 = mybir.dt.float32
    F32R = mybir.dt.float32r
    EXP = mybir.ActivationFunctionType.Exp

    nc = bacc.Bacc(get_trn_type() or "TRN2", target_bir_lowering=False, debug=False)

    # ---- DRAM parameters (per core) ----
    xT = nc.declare_dram_parameter("xT", [D, S], F32R, isOutput=False)        # x[b].T
    wq = nc.declare_dram_parameter("wq", [D, DV], F32R, isOutput=False)       # (W_q,g / 8).T
    wk = nc.declare_dram_parameter("wk", [D, DV], F32R, isOutput=False)       # W_k,g.T
    wv = nc.declare_dram_parameter("wv", [D, DV], F32R, isOutput=False)       # W_v,g.T
    ow = nc.declare_dram_parameter("ow", [DV, D], F32R, isOutput=False)       # W_out[:, g].T
    bq = nc.declare_dram_parameter("bq", [DV, 1], F32, isOutput=False)        # q bias / 8
    bk = nc.declare_dram_parameter("bk", [DV, 1], F32, isOutput=False)
    kb = nc.declare_dram_parameter("kb", [S, 1], F32, isOutput=False)         # key-mask bias
    y = nc.declare_dram_parameter("y", [S, D], F32, isOutput=True)            # partial output

    with tile.TileContext(nc) as tc, ExitStack() as octx:
        # ---- persistent SBUF ----
        pers = octx.enter_context(tc.tile_pool(name="pers", bufs=1))
        kT = [pers.tile([128, S], F32R, tag=f"kT{p}", name=f"kT{p}") for p in range(4)]
        vx = [pers.tile([128, HPC * 65], F32R, tag=f"vx{j}", name=f"vx{j}") for j in range(NJT)]
        bq_t = pers.tile([128, 4], F32, tag="bq")
        bk_t = pers.tile([128, 4], F32, tag="bk")
        kb_t = pers.tile([128, NJT], F32, tag="kb")
        cmt = pers.tile([128, 128], F32, tag="cmt")   # triangular boundary mask

        nc.sync.dma_start(bq_t[:], bq[:].squeeze(1).rearrange("(t p) -> p t", p=128))
        nc.sync.dma_start(bk_t[:], bk[:].squeeze(1).rearrange("(t p) -> p t", p=128))
        nc.sync.dma_start(kb_t[:], kb[:].squeeze(1).rearrange("(t p) -> p t", p=128))

        # keep (0) iff c - pj >= 0, else -1e30  (boundary block: col c = local
        # query offset, partition pj = key offset within the diagonal block)
        nc.vector.memset(cmt[:], 0.0)
        nc.gpsimd.affine_select(
            out=cmt[:], in_=cmt[:], compare_op=mybir.AluOpType.is_ge,
            fill=-1e30, base=0, pattern=[[1, 128]], channel_multiplier=-1,
        )

        # ones columns of vx tiles (col 64 of each 65-wide head slot)
        for j in range(NJT):
            ones_view = vx[j][:].bitcast(F32).rearrange("p (h c) -> p h c", c=65)[:, :, 64:65]
            nc.vector.memset(ones_view, 1.0)

        # rotating pools
        wpool = octx.enter_context(tc.tile_pool(name="wpool", bufs=1))
        qwin = octx.enter_context(tc.tile_pool(name="qwin", bufs=QWB))
        anwin = octx.enter_context(tc.tile_pool(name="anwin", bufs=AWB))
        xpool = octx.enter_context(tc.tile_pool(name="xpool", bufs=XPB))
        epool = octx.enter_context(tc.tile_pool(name="epool", bufs=EPB))
        npool = octx.enter_context(tc.tile_pool(name="npool", bufs=NPB))
        ypool = octx.enter_context(tc.tile_pool(name="ypool", bufs=YPB))
        ps = octx.enter_context(tc.tile_pool(name="ps", bufs=1, space="PSUM"))
        aps = octx.enter_context(tc.tile_pool(name="aps", bufs=APSB, space="PSUM"))

        for _rep in range(KREP):
            # ---- weights + first x block, interleaved so the first q psum
            # group unblocks as early as possible ----
            wq_t = [wpool.tile([128, DV], F32R, tag=f"wq{d}", name=f"wq{d}") for d in range(8)]
            wk_t = [wpool.tile([128, DV], F32R, tag=f"wk{d}", name=f"wk{d}") for d in range(8)]
            wv_t = [wpool.tile([128, DV], F32R, tag=f"wv{d}", name=f"wv{d}") for d in range(8)]
            ow_t = [wpool.tile([128, SB], F32R, tag=f"ow{i}", name=f"ow{i}") for i in range(8)]
            xt0 = []
            for d in range(8):
                nc.sync.dma_start(wq_t[d][:], wq[128 * d:128 * d + 128, :])
                t = xpool.tile([128, SB], F32R, tag="xt")
                nc.sync.dma_start(t[:], xT[128 * d:128 * d + 128, 0:SB])
                xt0.append(t)
            for d in range(8):
                nc.sync.dma_start(wk_t[d][:], wk[128 * d:128 * d + 128, :])
            for d in range(8):
                nc.sync.dma_start(wv_t[d][:], wv[128 * d:128 * d + 128, :])
            for p in range(4):
                for ot in range(2):
                    nc.sync.dma_start(ow_t[2 * p + ot][:],
                                      ow[128 * p:128 * p + 128, SB * ot:SB * ot + SB])

            for sblk in range(NSB):
                ssl = slice(SB * sblk, SB * sblk + SB)
                # ---- QKV projection for this s-block ----
                if sblk == 0:
                    xt = xt0
                else:
                    xt = []
                    for d in range(8):
                        t = xpool.tile([128, SB], F32R, tag="xt")
                        nc.sync.dma_start(t[:], xT[128 * d:128 * d + 128, ssl])
                        xt.append(t)
                qw = [qwin.tile([128, SB], F32R, tag=f"qw{o}", name=f"qw{o}")
                      for o in range(4)]
                for o in range(4):
                    osl = slice(128 * o, 128 * o + 128)
                    pq = ps.tile([128, SB], F32, tag="psq", bufs=PQB)
                    for d in range(8):
                        nc.tensor.matmul(pq[:], wq_t[d][:, osl], xt[d][:],
                                         start=(d == 0), stop=(d == 7))
                    nc.vector.tensor_scalar_add(qw[o][:], pq[:], bq_t[:, o:o + 1])
                for ssub in range(4):
                    jt = 4 * sblk + ssub
                    pv = ps.tile([128, SB], F32, tag="psq", bufs=PQB)
                    for d in range(8):
                        nc.tensor.matmul(pv[:], xt[d][:, 128 * ssub:128 * ssub + 128],
                                         wv_t[d][:], start=(d == 0), stop=(d == 7))
                    dst = vx[jt][:].rearrange("p (h c) -> p h c", c=65)[:, :, 0:64]
                    src = pv[:].rearrange("p (h c) -> p h c", c=64)
                    nc.vector.tensor_copy(dst, src)
                for o in range(4):
                    osl = slice(128 * o, 128 * o + 128)
                    pk = ps.tile([128, SB], F32, tag="psq", bufs=PQB)
                    for d in range(8):
                        nc.tensor.matmul(pk[:], wk_t[d][:, osl], xt[d][:],
                                         start=(d == 0), stop=(d == 7))
                    nc.vector.tensor_scalar_add(kT[o][:, ssl], pk[:], bk_t[:, o:o + 1])

                # ---- attention for i-window sblk ----
                i0 = SB * sblk
                an = [anwin.tile([128, SB], F32R, tag=f"an{o}", name=f"an{o}")
                      for o in range(4)]
                njt = 4 * sblk + 4
                for p in range(4):
                    pa = aps.tile([65, SB], F32, tag="aps")
                    pb = aps.tile([65, SB], F32, tag="aps")
                    for jt in range(njt):
                        jsl = slice(128 * jt, 128 * jt + 128)
                        r = jt - 4 * sblk          # negative: off-diagonal
                        c0 = 128 * r if r > 0 else 0   # first valid col in i-window
                        c0p = min(c0, SB - 256)        # matmul left edge (N >= 256)
                        w = SB - c0p
                        sA = ps.tile([128, w], F32, tag="pss", bufs=PSB)
                        sB = ps.tile([128, w], F32, tag="pss", bufs=PSB)
                        nc.tensor.matmul(sA[:], kT[p][0:64, jsl],
                                         qw[p][0:64, c0p:SB], start=True, stop=True)
                        nc.tensor.matmul(sB[:], kT[p][64:128, jsl],
                                         qw[p][64:128, c0p:SB], start=True, stop=True)
                        if r >= 0:  # triangular boundary block at window cols c0:c0+128
                            b0 = c0 - c0p
                            nc.vector.tensor_add(sA[:, b0:b0 + 128], sA[:, b0:b0 + 128], cmt[:])
                            nc.vector.tensor_add(sB[:, b0:b0 + 128], sB[:, b0:b0 + 128], cmt[:])
                        eA = epool.tile([128, SB], F32R, tag="e")
                        eB = epool.tile([128, SB], F32R, tag="e")
                        if c0 > c0p:
                            nc.gpsimd.memset(eA[:, c0p:c0].bitcast(F32), 0.0)
                            nc.gpsimd.memset(eB[:, c0p:c0].bitcast(F32), 0.0)
                        nc.scalar.activation(eA[:, c0:SB], sA[:, c0 - c0p:w], EXP,
                                             bias=kb_t[:, jt:jt + 1])
                        nc.scalar.activation(eB[:, c0:SB], sB[:, c0 - c0p:w], EXP,
                                             bias=kb_t[:, jt:jt + 1])
                        va = vx[jt][:, 65 * (2 * p):65 * (2 * p) + 65]
                        vb = vx[jt][:, 65 * (2 * p + 1):65 * (2 * p + 1) + 65]
                        nc.tensor.matmul(pa[:, c0p:SB], va, eA[:, c0p:SB],
                                         start=(jt == 0), stop=(jt == njt - 1))
                        nc.tensor.matmul(pb[:, c0p:SB], vb, eB[:, c0p:SB],
                                         start=(jt == 0), stop=(jt == njt - 1))
                    for ps_t, half in ((pa, 0), (pb, 1)):
                        rec = npool.tile([1, SB], F32, tag="rec")
                        nc.vector.reciprocal(rec[:], ps_t[64:65, :])
                        rb = npool.tile([64, SB], F32, tag="rb")
                        nc.gpsimd.partition_broadcast(rb[:], rec[:])
                        out = an[p][64 * half:64 * half + 64, :]
                        nc.vector.tensor_mul(out, ps_t[0:64, :], rb[:])

                # ---- output projection for this i-window ----
                for st in range(4):
                    lsl = slice(128 * st, 128 * st + 128)
                    gsl = slice(i0 + 128 * st, i0 + 128 * st + 128)
                    for ot in range(2):
                        py = ps.tile([128, SB], F32, tag="psj", bufs=PJB)
                        for p in range(4):
                            nc.tensor.matmul(py[:], an[p][:, lsl],
                                             ow_t[2 * p + ot][:],
                                             start=(p == 0), stop=(p == 3))
                        yt = ypool.tile([128, SB], F32, tag="yt")
                        nc.vector.tensor_copy(yt[:], py[:])
                        # y goes out on the gpsimd SWDGE queue so input DMAs
                        # on the SP queue never wait behind proj-dependent
                        # output transfers
                        nc.gpsimd.dma_start(y[gsl, SB * ot:SB * ot + SB], yt[:])

    nc.compile()
    return nc


def _get_module():
    if "nc" not in _CACHE:
        _CACHE["nc"] = _build_module()
    return _CACHE["nc"]


def _host_prep(x, mask, qkv_w, qkv_b, out_w):
    """Per-core input maps."""
    scale = np.float32(1.0 / np.sqrt(HD))
    in_maps = []
    for c in range(N_CORES):
        b, g = divmod(c, 2)
        qr = slice(g * DV, g * DV + DV)
        kr = slice(D + g * DV, D + g * DV + DV)
        vr = slice(2 * D + g * DV, 2 * D + g * DV + DV)
        in_maps.append({
            "xT": np.ascontiguousarray(x[b].T),
            "wq": np.ascontiguousarray(qkv_w[qr].T * scale),
            "wk": np.ascontiguousarray(qkv_w[kr].T),
            "wv": np.ascontiguousarray(qkv_w[vr].T),
            "ow": np.ascontiguousarray(out_w[:, g * DV:g * DV + DV].T),
            "bq": (qkv_b[qr] * scale).reshape(DV, 1).astype(np.float32),
            "bk": qkv_b[kr].reshape(DV, 1).astype(np.float32),
            "kb": np.where(mask[b] != 0, 0.0, -1e30).astype(np.float32).reshape(S, 1),
        })
    return in_maps


def _host_gather(results, qkv_b, out_b, out_w):
    # constant bias: out_b + W_out @ v_bias (v bias commutes through attention)
    bias = out_b + out_w @ qkv_b[2 * D:3 * D]
    y = np.empty((B, S, D), dtype=np.float32)
    for b in range(B):
        y[b] = results[2 * b]["y"] + results[2 * b + 1]["y"] + bias[None, :]
    return y


def kernel(x, mask, qkv_w, qkv_b, out_w, out_b):
    import time
    from concourse.bass_utils import run_bass_kernel_spmd

    nc = _get_module()
    in_maps = _host_prep(x, mask, qkv_w, qkv_b, out_w)
    last = None
    for attempt in range(3):
        try:
            res = run_bass_kernel_spmd(nc, in_maps, core_ids=list(range(N_CORES)))
            return _host_gather(res.results, qkv_b, out_b, out_w)
        except Exception as e:  # rare transient device faults: retry after recovery
            last = e
            time.sleep(10 * (attempt + 1))
    raise last


# revision 9
# speedup vs baseline: 1.7493x; 1.7493x over previous
"""Multi-head causal attention (B=4, S=2048, D=1024, H=16) on 8 TRN2 NeuronCores.

Sharding: core c -> (batch b = c//2, head-group g = c%2). Each core computes
8 heads for one batch: QKV projection (tensor-parallel column slice), causal
softmax attention, and a row-parallel slice of the output projection. The two
cores of a batch produce partial bf16 outputs that the host sums in fp32;
biases that commute with the attention (v bias, out bias) are folded into a
single host-side vector add.

Single fused pipeline per 512-token s-block: QKV(s) -> attention(i-window s)
-> output projection(s), with cross-block overlap (the ACT-engine exp of
window s runs under the QKV matmuls of window s+1). Three scheduling
enablers, found via the timeline cost-model sim:
 - separate PSUM tag rings for QKV groups (psq), score tiles (pss), proj
   groups (psj) and attn accumulators (aps) -- a shared ring FIFO-serializes
   the phases (QKV(s+1) psum allocation otherwise waits on proj(s));
 - y output DMA rides the gpsimd SWDGE queue so input x/weight DMAs on the
   SP queue never wait behind proj-dependent output transfers;
 - double-buffered weight tiles so rep-to-rep (KREP) execution pipelines.

All matmul operands are bf16 (fp32 PSUM accumulation; softmax normalization
in fp32). Measured rel err ~4e-3 vs fp32 reference (gate 2e-2). Layout:
 - q/k transposed: kT[p] = [128 partitions (2 heads x 64 hd), S]; q in a
   rotating per-window buffer qw[p] = [128, 512]. scoresT = kT.T @ qw has
   keys on partitions; the two heads of a pair run as concurrent row-tiled
   matmuls (tile_position (0,0)/(64,0) from base partitions).
 - v is produced in [s, dv] layout with an interleaved ones column per head
   ([v_h | 1], width 65) so attn@v also yields the softmax denominator row.
 - causal: off-diagonal j-tiles full 512-wide; diagonal j-tiles width
   max(256, 512-128r); the one triangular 128x128 boundary block is masked
   additively; invalid strip of the exp tile zero-filled on GpSimd (r=3
   only). Key (padding) mask enters as the per-partition exp bias.
 - softmax denominators: reciprocal on DVE, partition-broadcast on GpSimd,
   one multiply into the normalized attention window buffer.
"""

import numpy as np
from contextlib import ExitStack

B, S, D, H = 4, 2048, 1024, 16
HD = D // H          # 64
HPC = H // 2         # 8 heads per core
DV = HPC * HD        # 512 v-dims per core
N_CORES = 8
SB = 512             # i-tile width (matmul N)
NSB = S // SB        # 4
NJT = S // 128       # 16 j-tiles

_CACHE = {}


def _build_module():
    import os
    KREP = int(os.environ.get("KREP", "1"))
    PSB = int(os.environ.get("PSB", "3"))
    PJB = int(os.environ.get("PJB", "1"))
    PQB = int(os.environ.get("PQB", "2"))
    APSB = int(os.environ.get("APSB", "2"))
    EPB = int(os.environ.get("EPB", "8"))
    XPB = int(os.environ.get("XPB", "12"))
    NPB = int(os.environ.get("NPB", "4"))
    YPB = int(os.environ.get("YPB", "6"))
    QWB = int(os.environ.get("QWB", "2"))
    WPB = int(os.environ.get("WPB", "2"))
    AWB = int(os.environ.get("AWB", "2"))
    import concourse.bacc as bacc
    import concourse.mybir as mybir
    import concourse.tile as tile
    from concourse._compat import get_trn_type

    F32 = mybir.dt.float32
    F32R = mybir.dt.float32r
    BF16 = mybir.dt.bfloat16
    EXP = mybir.ActivationFunctionType.Exp

    nc = bacc.Bacc(get_trn_type() or "TRN2", target_bir_lowering=False, debug=False)

    # ---- DRAM parameters (per core) ----
    xT = nc.declare_dram_parameter("xT", [D, S], BF16, isOutput=False)        # x[b].T
    wq = nc.declare_dram_parameter("wq", [D, DV], BF16, isOutput=False)       # (W_q,g / 8).T
    wk = nc.declare_dram_parameter("wk", [D, DV], BF16, isOutput=False)       # W_k,g.T
    wv = nc.declare_dram_parameter("wv", [D, DV], BF16, isOutput=False)       # W_v,g.T
    ow = nc.declare_dram_parameter("ow", [DV, D], BF16, isOutput=False)       # W_out[:, g].T
    bq = nc.declare_dram_parameter("bq", [DV, 1], F32, isOutput=False)        # q bias / 8
    bk = nc.declare_dram_parameter("bk", [DV, 1], F32, isOutput=False)
    kb = nc.declare_dram_parameter("kb", [S, 1], F32, isOutput=False)         # key-mask bias
    y = nc.declare_dram_parameter("y", [S, D], BF16, isOutput=True)            # partial output

    with tile.TileContext(nc) as tc, ExitStack() as octx:
        # ---- persistent SBUF ----
        pers = octx.enter_context(tc.tile_pool(name="pers", bufs=1))
        kT = [pers.tile([128, S], BF16, tag=f"kT{p}", name=f"kT{p}") for p in range(4)]
        vx = [pers.tile([128, HPC * 65], BF16, tag=f"vx{j}", name=f"vx{j}") for j in range(NJT)]
        bq_t = pers.tile([128, 4], F32, tag="bq")
        bk_t = pers.tile([128, 4], F32, tag="bk")
        kb_t = pers.tile([128, NJT], F32, tag="kb")
        cmt = pers.tile([128, 128], F32, tag="cmt")   # triangular boundary mask

        nc.sync.dma_start(bq_t[:], bq[:].squeeze(1).rearrange("(t p) -> p t", p=128))
        nc.sync.dma_start(bk_t[:], bk[:].squeeze(1).rearrange("(t p) -> p t", p=128))
        nc.sync.dma_start(kb_t[:], kb[:].squeeze(1).rearrange("(t p) -> p t", p=128))

        # keep (0) iff c - pj >= 0, else -1e30  (boundary block: col c = local
        # query offset, partition pj = key offset within the diagonal block)
        nc.vector.memset(cmt[:], 0.0)
        nc.gpsimd.affine_select(
            out=cmt[:], in_=cmt[:], compare_op=mybir.AluOpType.is_ge,
            fill=-1e30, base=0, pattern=[[1, 128]], channel_multiplier=-1,
        )

        # ones columns of vx tiles (col 64 of each 65-wide head slot)
        for j in range(NJT):
            ones_view = vx[j][:].rearrange("p (h c) -> p h c", c=65)[:, :, 64:65]
            nc.vector.memset(ones_view, 1.0)

        # rotating pools
        wpool = octx.enter_context(tc.tile_pool(name="wpool", bufs=WPB))
        qwin = octx.enter_context(tc.tile_pool(name="qwin", bufs=QWB))
        anwin = octx.enter_context(tc.tile_pool(name="anwin", bufs=AWB))
        xpool = octx.enter_context(tc.tile_pool(name="xpool", bufs=XPB))
        epool = octx.enter_context(tc.tile_pool(name="epool", bufs=EPB))
        npool = octx.enter_context(tc.tile_pool(name="npool", bufs=NPB))
        ypool = octx.enter_context(tc.tile_pool(name="ypool", bufs=YPB))
        ps = octx.enter_context(tc.tile_pool(name="ps", bufs=1, space="PSUM"))
        aps = octx.enter_context(tc.tile_pool(name="aps", bufs=APSB, space="PSUM"))

        for _rep in range(KREP):
            # ---- weights + first x block, interleaved so the first q psum
            # group unblocks as early as possible ----
            wq_t = [wpool.tile([128, DV], BF16, tag=f"wq{d}", name=f"wq{d}") for d in range(8)]
            wk_t = [wpool.tile([128, DV], BF16, tag=f"wk{d}", name=f"wk{d}") for d in range(8)]
            wv_t = [wpool.tile([128, DV], BF16, tag=f"wv{d}", name=f"wv{d}") for d in range(8)]
            ow_t = [wpool.tile([128, SB], BF16, tag=f"ow{i}", name=f"ow{i}") for i in range(8)]
            xt0 = []
            for d in range(8):
                nc.sync.dma_start(wq_t[d][:], wq[128 * d:128 * d + 128, :])
                t = xpool.tile([128, SB], BF16, tag="xt")
                nc.sync.dma_start(t[:], xT[128 * d:128 * d + 128, 0:SB])
                xt0.append(t)
            for d in range(8):
                nc.sync.dma_start(wk_t[d][:], wk[128 * d:128 * d + 128, :])
            for d in range(8):
                nc.sync.dma_start(wv_t[d][:], wv[128 * d:128 * d + 128, :])
            for p in range(4):
                for ot in range(2):
                    nc.sync.dma_start(ow_t[2 * p + ot][:],
                                      ow[128 * p:128 * p + 128, SB * ot:SB * ot + SB])

            for sblk in range(NSB):
                ssl = slice(SB * sblk, SB * sblk + SB)
                # ---- QKV projection for this s-block ----
                if sblk == 0:
                    xt = xt0
                else:
                    xt = []
                    for d in range(8):
                        t = xpool.tile([128, SB], BF16, tag="xt")
                        nc.sync.dma_start(t[:], xT[128 * d:128 * d + 128, ssl])
                        xt.append(t)
                qw = [qwin.tile([128, SB], BF16, tag=f"qw{o}", name=f"qw{o}")
                      for o in range(4)]
                for o in range(4):
                    osl = slice(128 * o, 128 * o + 128)
                    pq = ps.tile([128, SB], F32, tag="psq", bufs=PQB)
                    for d in range(8):
                        nc.tensor.matmul(pq[:], wq_t[d][:, osl], xt[d][:],
                                         start=(d == 0), stop=(d == 7))
                    nc.vector.tensor_scalar_add(qw[o][:], pq[:], bq_t[:, o:o + 1])
                for ssub in range(4):
                    jt = 4 * sblk + ssub
                    pv = ps.tile([128, SB], F32, tag="psq", bufs=PQB)
                    for d in range(8):
                        nc.tensor.matmul(pv[:], xt[d][:, 128 * ssub:128 * ssub + 128],
                                         wv_t[d][:], start=(d == 0), stop=(d == 7))
                    dst = vx[jt][:].rearrange("p (h c) -> p h c", c=65)[:, :, 0:64]
                    src = pv[:].rearrange("p (h c) -> p h c", c=64)
                    nc.vector.tensor_copy(dst, src)
                for o in range(4):
                    osl = slice(128 * o, 128 * o + 128)
                    pk = ps.tile([128, SB], F32, tag="psq", bufs=PQB)
                    for d in range(8):
                        nc.tensor.matmul(pk[:], wk_t[d][:, osl], xt[d][:],
                                         start=(d == 0), stop=(d == 7))
                    nc.vector.tensor_scalar_add(kT[o][:, ssl], pk[:], bk_t[:, o:o + 1])

                # ---- attention for i-window sblk ----
                i0 = SB * sblk
                an = [anwin.tile([128, SB], BF16, tag=f"an{o}", name=f"an{o}")
                      for o in range(4)]
                njt = 4 * sblk + 4
                for p in range(4):
                    pa = aps.tile([65, SB], F32, tag="aps")
                    pb = aps.tile([65, SB], F32, tag="aps")
                    for jt in range(njt):
                        jsl = slice(128 * jt, 128 * jt + 128)
                        r = jt - 4 * sblk          # negative: off-diagonal
                        c0 = 128 * r if r > 0 else 0   # first valid col in i-window
                        c0p = min(c0, SB - 256)        # matmul left edge (N >= 256)
                        w = SB - c0p
                        sA = ps.tile([128, w], F32, tag="pss", bufs=PSB)
                        sB = ps.tile([128, w], F32, tag="pss", bufs=PSB)
                        nc.tensor.matmul(sA[:], kT[p][0:64, jsl],
                                         qw[p][0:64, c0p:SB], start=True, stop=True)
                        nc.tensor.matmul(sB[:], kT[p][64:128, jsl],
                                         qw[p][64:128, c0p:SB], start=True, stop=True)
                        if r >= 0:  # triangular boundary block at window cols c0:c0+128
                            b0 = c0 - c0p
                            nc.vector.tensor_add(sA[:, b0:b0 + 128], sA[:, b0:b0 + 128], cmt[:])
                            nc.vector.tensor_add(sB[:, b0:b0 + 128], sB[:, b0:b0 + 128], cmt[:])
                        eA = epool.tile([128, SB], BF16, tag="e")
                        eB = epool.tile([128, SB], BF16, tag="e")
                        if c0 > c0p:
                            nc.gpsimd.memset(eA[:, c0p:c0], 0.0)
                            nc.gpsimd.memset(eB[:, c0p:c0], 0.0)
                        nc.scalar.activation(eA[:, c0:SB], sA[:, c0 - c0p:w], EXP,
                                             bias=kb_t[:, jt:jt + 1])
                        nc.scalar.activation(eB[:, c0:SB], sB[:, c0 - c0p:w], EXP,
                                             bias=kb_t[:, jt:jt + 1])
                        va = vx[jt][:, 65 * (2 * p):65 * (2 * p) + 65]
                        vb = vx[jt][:, 65 * (2 * p + 1):65 * (2 * p + 1) + 65]
                        nc.tensor.matmul(pa[:, c0p:SB], va, eA[:, c0p:SB],
                                         start=(jt == 0), stop=(jt == njt - 1))
                        nc.tensor.matmul(pb[:, c0p:SB], vb, eB[:, c0p:SB],
                                         start=(jt == 0), stop=(jt == njt - 1))
                    for ps_t, half in ((pa, 0), (pb, 1)):
                        rec = npool.tile([1, SB], F32, tag="rec")
                        nc.vector.reciprocal(rec[:], ps_t[64:65, :])
                        rb = npool.tile([64, SB], F32, tag="rb")
                        nc.gpsimd.partition_broadcast(rb[:], rec[:])
                        out = an[p][64 * half:64 * half + 64, :]
                        nc.vector.tensor_mul(out, ps_t[0:64, :], rb[:])

                # ---- output projection for this i-window ----
                for st in range(4):
                    lsl = slice(128 * st, 128 * st + 128)
                    gsl = slice(i0 + 128 * st, i0 + 128 * st + 128)
                    for ot in range(2):
                        py = ps.tile([128, SB], F32, tag="psj", bufs=PJB)
                        for p in range(4):
                            nc.tensor.matmul(py[:], an[p][:, lsl],
                                             ow_t[2 * p + ot][:],
                                             start=(p == 0), stop=(p == 3))
                        yt = ypool.tile([128, SB], BF16, tag="yt")
                        nc.vector.tensor_copy(yt[:], py[:])
                        # y goes out on the gpsimd SWDGE queue so input DMAs
                        # on the SP queue never wait behind proj-dependent
                        # output transfers
                        nc.gpsimd.dma_start(y[gsl, SB * ot:SB * ot + SB], yt[:])

    nc.compile()
    return nc


def _get_module():
    if "nc" not in _CACHE:
        _CACHE["nc"] = _build_module()
    return _CACHE["nc"]


def _host_prep(x, mask, qkv_w, qkv_b, out_w):
    """Per-core input maps."""
    scale = np.float32(1.0 / np.sqrt(HD))
    in_maps = []
    for c in range(N_CORES):
        b, g = divmod(c, 2)
        qr = slice(g * DV, g * DV + DV)
        kr = slice(D + g * DV, D + g * DV + DV)
        vr = slice(2 * D + g * DV, 2 * D + g * DV + DV)
        import ml_dtypes
        bf = lambda a: np.ascontiguousarray(a).astype(ml_dtypes.bfloat16)
        in_maps.append({
            "xT": bf(x[b].T),
            "wq": bf(qkv_w[qr].T * scale),
            "wk": bf(qkv_w[kr].T),
            "wv": bf(qkv_w[vr].T),
            "ow": bf(out_w[:, g * DV:g * DV + DV].T),
            "bq": (qkv_b[qr] * scale).reshape(DV, 1).astype(np.float32),
            "bk": qkv_b[kr].reshape(DV, 1).astype(np.float32),
            "kb": np.where(mask[b] != 0, 0.0, -1e30).astype(np.float32).reshape(S, 1),
        })
    return in_maps


def _host_gather(results, qkv_b, out_b, out_w):
    # constant bias: out_b + W_out @ v_bias (v bias commutes through attention)
    bias = out_b + out_w @ qkv_b[2 * D:3 * D]
    y = np.empty((B, S, D), dtype=np.float32)
    for b in range(B):
        y[b] = (results[2 * b]["y"].astype(np.float32)
                + results[2 * b + 1]["y"].astype(np.float32) + bias[None, :])
    return y


def kernel(x, mask, qkv_w, qkv_b, out_w, out_b):
    import time
    from concourse.bass_utils import run_bass_kernel_spmd

    nc = _get_module()
    in_maps = _host_prep(x, mask, qkv_w, qkv_b, out_w)
    last = None
    for attempt in range(3):
        try:
            res = run_bass_kernel_spmd(nc, in_maps, core_ids=list(range(N_CORES)))
            return _host_gather(res.results, qkv_b, out_b, out_w)
        except Exception as e:  # rare transient device faults: retry after recovery
            last = e
            time.sleep(10 * (attempt + 1))
    raise last
